# revision 11
# baseline (speedup 1.0000x reference)
"""BLSTM-CRF NLL on 8 Trainium2 NeuronCores.

Sharding: data-parallel over batch (4 sequences per core); everything for
those 4 sequences (embedding gather, BiLSTM, emissions, CRF forward pass)
runs fused in one Bass kernel per core. Host does only index prep, the
input-independent pieces of the gold score, and the final log/sum.

LSTM formulation: sigmoid(x) = 0.5*tanh(x/2)+0.5 with the 0.5 folded into
the i/f/o rows of the weights, so the only ACT functions used are tanh and
(in the CRF tail) exp - both live in the same activation table set.
CRF runs in probability space: a_{t+1} = (a_t @ (exp(trans)/32)) * exp(emit),
with the 511*log(32) rescale added back on the host.
"""
import os
import numpy as np
import ml_dtypes

B, T = 32, 512
WV, CV = 32000, 8000
DW, DC = 128, 64
D = DW + DC
H = 128
G = 4 * H
L = 32
NCORES = 8
BL = B // NCORES          # 4 sequences per core
NFLAT = BL * T            # 2048 (b, t) pairs per core, b-major
NT = int(os.environ.get("BLSTM_NT", str(T)))  # steps (512; smaller for tests)

_CACHE = {}

bf16 = ml_dtypes.bfloat16


def _build(nc_mod):
    import concourse.bass as bass
    import concourse.mybir as mybir
    from concourse.tile import TileContext
    from concourse.masks import make_identity

    f32 = mybir.dt.float32
    b16 = mybir.dt.bfloat16
    i32 = mybir.dt.int32
    Tanh = mybir.ActivationFunctionType.Tanh
    Exp = mybir.ActivationFunctionType.Exp

    nc = nc_mod.Bacc()
    wemb_d = nc.declare_dram_parameter("wemb", [WV, DW], b16, isOutput=False)
    cemb_d = nc.declare_dram_parameter("cemb", [CV, DC], b16, isOutput=False)
    idx_d = nc.declare_dram_parameter("idx", [128, 32], i32, isOutput=False)
    ihA_d = nc.declare_dram_parameter("ihA", [2, 4, 128, 128], b16, isOutput=False)
    ihB_d = nc.declare_dram_parameter("ihB", [2, 4, 65, 128], b16, isOutput=False)
    whh_d = nc.declare_dram_parameter("whh", [2, 4, 128, 128], b16, isOutput=False)
    wo_d = nc.declare_dram_parameter("wo", [2, 128, 32], b16, isOutput=False)
    sml_d = nc.declare_dram_parameter("sml", [32, 35], f32, isOutput=False)
    oneh_d = nc.declare_dram_parameter("oneh", [32, NT * BL], f32, isOutput=False)
    out_d = nc.declare_dram_parameter("out", [1, 2 * BL], f32, isOutput=True)

    NF = NT * BL          # flat (b, t) count actually processed
    KT = NF // 128        # gather tiles (16 at NT=512)

    with TileContext(nc) as tc:
        with tc.tile_pool(name="cst", bufs=1) as cst, \
             tc.tile_pool(name="ps", bufs=2, space="PSUM") as ps, \
             tc.tile_pool(name="psg", bufs=3, space="PSUM") as psg, \
             tc.tile_pool(name="sb", bufs=3) as sb:

            # ---------- constants / weights in ----------
            ident = cst.tile([128, 128], b16)
            make_identity(nc, ident[:])
            idx_sb = cst.tile([128, 32], i32)
            nc.sync.dma_start(out=idx_sb[:], in_=idx_d[:])
            sml = cst.tile([32, 35], f32)
            nc.sync.dma_start(out=sml[:], in_=sml_d[:])
            ihA, ihB, Whh, Wo = {}, {}, {}, {}
            for di in range(2):
                for j in range(4):
                    wa = cst.tile([128, 128], b16, tag=f"ihA{di}{j}")
                    nc.sync.dma_start(out=wa[:], in_=ihA_d[di, j])
                    ihA[di, j] = wa
                    wb = cst.tile([65, 128], b16, tag=f"ihB{di}{j}")
                    nc.sync.dma_start(out=wb[:], in_=ihB_d[di, j])
                    ihB[di, j] = wb
                    wh = cst.tile([128, 128], b16, tag=f"whh{di}{j}")
                    nc.sync.dma_start(out=wh[:], in_=whh_d[di, j])
                    Whh[di, j] = wh
                wo = cst.tile([128, 32], b16, tag=f"wo{di}")
                nc.sync.dma_start(out=wo[:], in_=wo_d[di])
                Wo[di] = wo

            # ---------- embedding gather + transpose ----------
            xT_hi = cst.tile([128, NF], b16)   # word dims 0:128, x^T
            xT_lo = cst.tile([65, NF], b16)    # char dims + ones row
            nc.gpsimd.memset(xT_lo[64:65, :], 1.0)
            for k in range(KT):
                xw = sb.tile([128, DW], b16, tag="xw")
                nc.gpsimd.indirect_dma_start(
                    out=xw[:], out_offset=None, in_=wemb_d[:],
                    in_offset=bass.IndirectOffsetOnAxis(
                        ap=idx_sb[:, k:k + 1], axis=0))
                tw = ps.tile([128, 128], b16, tag="big")
                nc.tensor.transpose(out=tw[:], in_=xw[:], identity=ident[:])
                nc.vector.tensor_copy(out=xT_hi[:, k * 128:(k + 1) * 128],
                                      in_=tw[:])
                xc = sb.tile([128, DC], b16, tag="xc")
                nc.gpsimd.indirect_dma_start(
                    out=xc[:], out_offset=None, in_=cemb_d[:],
                    in_offset=bass.IndirectOffsetOnAxis(
                        ap=idx_sb[:, 16 + k:17 + k], axis=0))
                tcp = ps.tile([64, 128], b16, tag="big")
                nc.tensor.transpose(out=tcp[:], in_=xc[:], identity=ident[:])
                nc.scalar.copy(out=xT_lo[0:64, k * 128:(k + 1) * 128],
                               in_=tcp[:])

            # ---------- xp GEMM:  xp^T[(chunk j), t, b] per dir ----------
            # xp buffer layout per dir: col = t*16 + j*4 + b  (bwd: t reversed)
            xp = {}
            for di in range(2):
                xpt = cst.tile([128, NT * 16], b16, tag=f"xp{di}")
                xp[di] = xpt
            for di in range(2):
                for j in range(4):
                    for b in range(BL):
                        gps = ps.tile([128, NT], f32, tag="big")
                        nc.tensor.matmul(gps[:], ihA[di, j][:],
                                         xT_hi[:, b * NT:(b + 1) * NT],
                                         start=True, stop=False)
                        nc.tensor.matmul(gps[:], ihB[di, j][:],
                                         xT_lo[:, b * NT:(b + 1) * NT],
                                         start=False, stop=True)
                        dst = xp[di][:].rearrange(
                            "p (t x) -> p t x", x=16)[:, :, j * 4 + b:j * 4 + b + 1]
                        if (j + di) % 2 == 0:
                            nc.vector.tensor_copy(out=dst, in_=gps[:].rearrange(
                                "p (t o) -> p t o", o=1))
                        else:
                            nc.scalar.copy(out=dst, in_=gps[:].rearrange(
                                "p (t o) -> p t o", o=1))

            # ---------- BiLSTM chains (lockstep f/b) ----------
            hh = {}
            for di in range(2):
                hht = cst.tile([128, NT * 4], b16, tag=f"hh{di}")
                hh[di] = hht
            h0 = cst.tile([128, 8], b16)
            nc.gpsimd.memset(h0[:], 0.0)
            cst8, acc = {}, {}
            for di in range(2):
                c_ = cst.tile([128, 4], f32, tag=f"c{di}")
                a_ = cst.tile([128, 1], f32, tag=f"acc{di}")
                nc.gpsimd.memset(c_[:], 0.0)
                cst8[di], acc[di] = c_, a_

            gsave = {}
            for di in range(2):
                g_ps = psg.tile([128, 16], f32, tag=f"g{di}")
                nc.tensor.matmul(g_ps[:], ident[:], xp[di][:, 0:16]
                                 if di == 0 else
                                 xp[di][:, (NT - 1) * 16:NT * 16],
                                 start=True, stop=False, skip_group_check=True)
                gsave[di] = g_ps
            for s in range(NT):
                for di in range(2):
                    # hh slot for this step (bwd history stored in natural t)
                    wr = s if di == 0 else (NT - 1 - s)
                    g_ps = gsave[di]
                    if s == 0:
                        hp = h0[:, di * 4:(di + 1) * 4]
                    else:
                        pw = (s - 1) if di == 0 else (NT - s)
                        hp = hh[di][:, pw * 4:(pw + 1) * 4]
                    for j in range(4):
                        nc.tensor.matmul(g_ps[:, j * 4:(j + 1) * 4],
                                         Whh[di, j][:], hp,
                                         start=False, stop=(j == 3),
                                         skip_group_check=True)
                    if s + 1 < NT:
                        xs2 = (s + 1) if di == 0 else (NT - 2 - s)
                        g_nx = psg.tile([128, 16], f32, tag=f"g{di}")
                        nc.tensor.matmul(g_nx[:], ident[:],
                                         xp[di][:, xs2 * 16:(xs2 + 1) * 16],
                                         start=True, stop=False,
                                         skip_group_check=True)
                        gsave[di] = g_nx
                    v = sb.tile([128, 16], f32, tag=f"v{di}")
                    nc.scalar.activation(v[:], g_ps[:], Tanh)
                    sg = sb.tile([128, 12], f32, tag=f"sg{di}")
                    nc.vector.tensor_scalar(
                        out=sg[:], in0=v[:, 0:12], scalar1=0.5, scalar2=0.5,
                        op0=mybir.AluOpType.mult, op1=mybir.AluOpType.add)
                    p_ = sb.tile([128, 4], f32, tag=f"p{di}")
                    nc.vector.tensor_mul(out=p_[:], in0=sg[:, 4:8],
                                         in1=cst8[di][:])
                    q_ = sb.tile([128, 4], f32, tag=f"q{di}")
                    nc.vector.tensor_mul(out=q_[:], in0=sg[:, 0:4],
                                         in1=v[:, 12:16])
                    nc.vector.tensor_add(out=cst8[di][:], in0=p_[:],
                                         in1=q_[:])
                    t_ = sb.tile([128, 4], f32, tag=f"t{di}")
                    nc.scalar.activation(t_[:], cst8[di][:], Tanh)
                    nc.vector.tensor_mul(out=hh[di][:, wr * 4:(wr + 1) * 4],
                                         in0=sg[:, 8:12], in1=t_[:])

            # ---------- emissions ----------
            emit_sb = cst.tile([32, NF], f32)
            em = cst.tile([32, NF], f32)
            CH = min(512, NF)
            for c0 in range(0, NF, CH):
                sl = slice(c0, c0 + CH)
                pse = ps.tile([32, CH], f32, tag="big")
                nc.tensor.matmul(pse[:], Wo[0][:], hh[0][:, sl],
                                 start=True, stop=False)
                nc.tensor.matmul(pse[:], Wo[1][:], hh[1][:, sl],
                                 start=False, stop=True)
                nc.vector.tensor_copy(out=emit_sb[:, sl], in_=pse[:])
                if c0 == 0:
                    nc.scalar.activation(em[:, 0:4], pse[:, 0:4], Exp,
                                         bias=sml[:, 33:34])
                    nc.scalar.activation(em[:, 4:CH], pse[:, 4:], Exp,
                                         bias=sml[:, 32:33])
                else:
                    nc.scalar.activation(em[:, sl], pse[:], Exp,
                                         bias=sml[:, 32:33])

            # ---------- gold emit score:  sum_t emit[y] per b ----------
            oneh = cst.tile([32, NF], f32)
            nc.sync.dma_start(out=oneh[:], in_=oneh_d[:])
            es = cst.tile([32, BL], f32)
            junk = cst.tile([32, NT], f32)
            ev = emit_sb[:].rearrange("p (t b) -> p b t", b=BL)
            ov = oneh[:].rearrange("p (t b) -> p b t", b=BL)
            for b in range(BL):
                nc.vector.affine_mul_reduce(
                    out=junk[:], accum_out=es[:, b:b + 1],
                    in0=ev[:, b], in1=ov[:, b], scale=1.0, bias=0.0)

            # ---------- CRF forward scan (prob domain, bf16 matmul) ----------
            Eb = cst.tile([32, 32], b16)
            nc.vector.tensor_copy(out=Eb[:], in_=sml[:, 0:32])
            aT = cst.tile([32, 4], b16)
            nc.vector.tensor_copy(out=aT[:], in_=em[:, 0:4])
            for t in range(1, NT):
                ca = psg.tile([32, 4], f32, tag="g0")
                nc.tensor.matmul(ca[:], Eb[:], aT[:],
                                 start=True, stop=True)
                a2 = cst.tile([32, 4], b16, tag=f"aT{t % 2}")
                nc.vector.tensor_mul(out=a2[:], in0=ca[:],
                                     in1=em[:, t * 4:(t + 1) * 4])
                aT = a2

            # ---------- outputs: vsum + emit score ----------
            vfin = cst.tile([32, 4], f32)
            nc.vector.tensor_mul(out=vfin[:], in0=aT[:],
                                 in1=sml[:, 34:35].to_broadcast([32, 4]))
            ones = cst.tile([32, 1], f32)
            nc.gpsimd.memset(ones[:], 1.0)
            outp0 = psg.tile([1, BL], f32, tag="g0")
            nc.tensor.matmul(outp0[:], ones[:], vfin[:], start=True, stop=True)
            outp1 = psg.tile([1, BL], f32, tag="g0")
            nc.tensor.matmul(outp1[:], ones[:], es[:], start=True, stop=True)
            out_sb = cst.tile([1, 2 * BL], f32)
            nc.vector.tensor_copy(out=out_sb[:, 0:BL], in_=outp0[:])
            nc.vector.tensor_copy(out=out_sb[:, BL:], in_=outp1[:])
            nc.gpsimd.dma_start(out=out_d[:], in_=out_sb[:])

    nc.compile()
    return nc


def _get_nc():
    if "nc" not in _CACHE:
        import concourse.bacc as bacc
        _CACHE["nc"] = _build(bacc)
    return _CACHE["nc"]


def _prep_weights(W_ih, b, W_hh, W_out):
    """Per-direction weight prep: gate order (i,f,o,g), i/f/o pre-scaled by
    0.5 for the tanh-half-angle sigmoid."""
    Wi = W_ih.astype(np.float64).copy()
    Wh = W_hh.astype(np.float64).copy()
    bb = b.astype(np.float64).copy()
    # torch gate order i,f,g,o -> ours i,f,o,g
    perm = np.concatenate([np.arange(0, 128), np.arange(128, 256),
                           np.arange(384, 512), np.arange(256, 384)])
    Wi, Wh, bb = Wi[perm], Wh[perm], bb[perm]
    scale = np.ones((512, 1)); scale[:384] = 0.5
    Wi *= scale; Wh *= scale; bb *= scale[:, 0]
    # ihA[j] = Wi[j*128:(j+1)*128, 0:128].T ; ihB[j] = [Wi[:,128:192] | b].T
    ihA = np.stack([Wi[j * 128:(j + 1) * 128, 0:128].T for j in range(4)])
    ihB = np.stack([np.concatenate([Wi[j * 128:(j + 1) * 128, 128:192],
                                    bb[j * 128:(j + 1) * 128, None]], 1).T
                    for j in range(4)])
    whh = np.stack([Wh[j * 128:(j + 1) * 128, :].T for j in range(4)])
    return (ihA.astype(bf16), ihB.astype(bf16), whh.astype(bf16))


def _run_device(word, char, wordemb, charemb, W_ih_f, W_hh_f, b_f,
                W_ih_b, W_hh_b, b_b, W_out, b_out, trans, start, end, y):
    from concourse.bass_utils import run_bass_kernel_spmd
    nc = _get_nc()

    ihA_f, ihB_f, whh_f = _prep_weights(W_ih_f, b_f, W_hh_f, W_out)
    ihA_b, ihB_b, whh_b = _prep_weights(W_ih_b, b_b, W_hh_b, W_out)
    ihA = np.stack([ihA_f, ihA_b]); ihB = np.stack([ihB_f, ihB_b])
    whh = np.stack([whh_f, whh_b])
    wo = np.stack([W_out[:, 0:128].T.astype(bf16),
                   W_out[:, 128:256].T.astype(bf16)])
    sml = np.zeros((32, 35), np.float32)
    sml[:, 0:32] = np.exp(trans.astype(np.float64) / 1.0).astype(np.float32) / 32.0
    sml[:, 32] = b_out
    sml[:, 33] = start + b_out
    sml[:, 34] = np.exp(end)
    wemb = wordemb.astype(bf16)
    cemb = charemb.astype(bf16)

    in_maps = []
    for c in range(NCORES):
        wl = word[c * BL:(c + 1) * BL, :NT]          # [4, NT]
        cl = char[c * BL:(c + 1) * BL, :NT]
        yl = y[c * BL:(c + 1) * BL, :NT]
        # column-major per gather call: call k covers flat (b-major) 128k..
        iw = wl.reshape(-1)          # n = b*NT + t
        ic = cl.reshape(-1)
        idx = np.zeros((128, 32), np.int32)
        for k in range(NT * BL // 128):
            idx[:, k] = iw[k * 128:(k + 1) * 128]
            idx[:, 16 + k] = ic[k * 128:(k + 1) * 128]
        oneh = np.zeros((32, NT * BL), np.float32)
        tt, bb2 = np.meshgrid(np.arange(NT), np.arange(BL), indexing="ij")
        oneh[yl.T.reshape(-1), (tt * BL + bb2).reshape(-1)] = 1.0
        in_maps.append(dict(wemb=wemb, cemb=cemb, idx=idx, ihA=ihA, ihB=ihB,
                            whh=whh, wo=wo, sml=sml, oneh=oneh))
    res = run_bass_kernel_spmd(nc, in_maps, list(range(NCORES)),
                               **_CACHE.get("run_kwargs", {}))
    _CACHE["last_res"] = res
    outs = [np.asarray(res.results[c]["out"], np.float64).reshape(2, BL)
            for c in range(NCORES)]
    return outs


def kernel(**inputs):
    word = np.asarray(inputs["word"]).astype(np.int64)
    char = np.asarray(inputs["char"]).astype(np.int64)
    y = np.asarray(inputs["y"]).astype(np.int64)
    wordemb = np.asarray(inputs["wordemb"], np.float32)
    charemb = np.asarray(inputs["charemb"], np.float32)
    W_ih_f = np.asarray(inputs["W_ih_f"], np.float32)
    W_hh_f = np.asarray(inputs["W_hh_f"], np.float32)
    b_f = np.asarray(inputs["b_f"], np.float32)
    W_ih_b = np.asarray(inputs["W_ih_b"], np.float32)
    W_hh_b = np.asarray(inputs["W_hh_b"], np.float32)
    b_b = np.asarray(inputs["b_b"], np.float32)
    W_out = np.asarray(inputs["W_out"], np.float32)
    b_out = np.asarray(inputs["b_out"], np.float32)
    trans = np.asarray(inputs["trans"], np.float32)
    start = np.asarray(inputs["start"], np.float32)
    end = np.asarray(inputs["end"], np.float32)

    mask = (char > 0)
    if not mask.all():
        return _host_fallback(**inputs)

    outs = _run_device(word, char, wordemb, charemb, W_ih_f, W_hh_f, b_f,
                       W_ih_b, W_hh_b, b_b, W_out, b_out, trans, start, end, y)

    total = 0.0
    tr64 = trans.astype(np.float64)
    for c in range(NCORES):
        vsum = outs[c][0]          # [4]
        es_dev = outs[c][1]        # [4] sum_t (W_out h2)[y]
        yl = y[c * BL:(c + 1) * BL]
        logZ = np.log(vsum) + (T - 1) * np.log(32.0)
        bout_sum = b_out.astype(np.float64)[yl].sum(1)
        trans_sc = tr64[yl[:, :-1], yl[:, 1:]].sum(1)
        gold = (start.astype(np.float64)[yl[:, 0]] + es_dev + bout_sum +
                trans_sc + end.astype(np.float64)[yl[:, -1]])
        total += np.sum(logZ - gold)
    return np.float32(total)


def _host_fallback(**inputs):
    """Pure numpy reference path (used only if masking assumptions break)."""
    word = np.asarray(inputs["word"]).astype(np.int64)
    char = np.asarray(inputs["char"]).astype(np.int64)
    y = np.asarray(inputs["y"]).astype(np.int64)
    wordemb = np.asarray(inputs["wordemb"], np.float32)
    charemb = np.asarray(inputs["charemb"], np.float32)
    W_out = np.asarray(inputs["W_out"], np.float32)
    b_out = np.asarray(inputs["b_out"], np.float32)
    trans = np.asarray(inputs["trans"], np.float32)
    start = np.asarray(inputs["start"], np.float32)
    end = np.asarray(inputs["end"], np.float32)

    def sig(x):
        return 1.0 / (1.0 + np.exp(-x))

    def lstm(xp, W_hh):
        h = np.zeros((B, H), np.float32)
        c = np.zeros((B, H), np.float32)
        hs = np.empty((T, B, H), np.float32)
        WT = W_hh.T
        for t in range(T):
            g = xp[t] + h @ WT
            i = sig(g[:, :H]); f = sig(g[:, H:2 * H])
            gg = np.tanh(g[:, 2 * H:3 * H]); o = sig(g[:, 3 * H:])
            c = f * c + i * gg
            h = o * np.tanh(c)
            hs[t] = h
        return hs

    mask = (char > 0).astype(np.float32)
    x = np.concatenate([wordemb[word], charemb[char]], -1)
    flat = x.reshape(-1, D)
    pf = (flat @ np.asarray(inputs["W_ih_f"], np.float32).T +
          np.asarray(inputs["b_f"], np.float32)).reshape(B, T, G)
    pb = (flat @ np.asarray(inputs["W_ih_b"], np.float32).T +
          np.asarray(inputs["b_b"], np.float32)).reshape(B, T, G)
    h_f = lstm(pf.transpose(1, 0, 2), np.asarray(inputs["W_hh_f"], np.float32))
    h_b = lstm(pb.transpose(1, 0, 2)[::-1],
               np.asarray(inputs["W_hh_b"], np.float32))[::-1]
    h = np.concatenate([h_f, h_b], -1).transpose(1, 0, 2)
    emit = (h.reshape(-1, 2 * H) @ W_out.T + b_out).reshape(B, T, L)
    emit = emit * mask[:, :, None]

    def lse(xx, axis):
        m = np.max(xx, axis=axis, keepdims=True)
        return np.squeeze(m, axis) + np.log(np.sum(np.exp(xx - m), axis=axis))

    alpha = start + emit[:, 0]
    for t in range(1, T):
        new = lse(alpha[:, :, None] + trans[None], 1) + emit[:, t]
        alpha = np.where(mask[:, t:t + 1] > 0, new, alpha)
    logZ = lse(alpha + end[None], 1)
    emit_sc = (np.take_along_axis(emit, y[:, :, None], 2)[..., 0] * mask).sum(1)
    trans_sc = (trans[y[:, :-1], y[:, 1:]] * mask[:, 1:]).sum(1)
    last = mask.sum(1).astype(np.int64) - 1
    y_last = y[np.arange(B), last]
    gold = start[y[:, 0]] + emit_sc + trans_sc + end[y_last]
    return np.float32(np.sum(logZ - gold))


# revision 12
# speedup vs baseline: 1.0231x; 1.0231x over previous
"""BLSTM-CRF NLL on 8 Trainium2 NeuronCores.

Sharding: data-parallel over batch (4 sequences per core); everything for
those 4 sequences (embedding gather, BiLSTM, emissions, CRF forward pass)
runs fused in one Bass kernel per core. Host does only index prep, the
input-independent pieces of the gold score, and the final log/sum.

LSTM formulation: sigmoid(x) = 0.5*tanh(x/2)+0.5 with the 0.5 folded into
the i/f/o rows of the weights, so the only ACT functions used are tanh and
(in the CRF tail) exp - both live in the same activation table set.
CRF runs in probability space: a_{t+1} = (a_t @ (exp(trans)/32)) * exp(emit),
with the 511*log(32) rescale added back on the host.
"""
import os
import numpy as np
import ml_dtypes

B, T = 32, 512
WV, CV = 32000, 8000
DW, DC = 128, 64
D = DW + DC
H = 128
G = 4 * H
L = 32
NCORES = 8
BL = B // NCORES          # 4 sequences per core
NFLAT = BL * T            # 2048 (b, t) pairs per core, b-major
NT = int(os.environ.get("BLSTM_NT", str(T)))  # steps (512; smaller for tests)

_CACHE = {}

bf16 = ml_dtypes.bfloat16


def _build(nc_mod):
    import concourse.bass as bass
    import concourse.mybir as mybir
    from concourse.tile import TileContext
    from concourse.masks import make_identity

    f32 = mybir.dt.float32
    b16 = mybir.dt.bfloat16
    i32 = mybir.dt.int32
    Tanh = mybir.ActivationFunctionType.Tanh
    Exp = mybir.ActivationFunctionType.Exp

    nc = nc_mod.Bacc()
    wemb_d = nc.declare_dram_parameter("wemb", [WV, DW], b16, isOutput=False)
    cemb_d = nc.declare_dram_parameter("cemb", [CV, DC], b16, isOutput=False)
    idx_d = nc.declare_dram_parameter("idx", [128, 32], i32, isOutput=False)
    ihA_d = nc.declare_dram_parameter("ihA", [2, 4, 128, 128], b16, isOutput=False)
    ihB_d = nc.declare_dram_parameter("ihB", [2, 4, 65, 128], b16, isOutput=False)
    whh_d = nc.declare_dram_parameter("whh", [2, 4, 128, 128], b16, isOutput=False)
    wo_d = nc.declare_dram_parameter("wo", [2, 128, 32], b16, isOutput=False)
    sml_d = nc.declare_dram_parameter("sml", [32, 35], f32, isOutput=False)
    oneh_d = nc.declare_dram_parameter("oneh", [32, NT * BL], f32, isOutput=False)
    out_d = nc.declare_dram_parameter("out", [1, 2 * BL], f32, isOutput=True)

    NF = NT * BL          # flat (b, t) count actually processed
    KT = NF // 128        # gather tiles (16 at NT=512)

    with TileContext(nc) as tc:
        with tc.tile_pool(name="cst", bufs=1) as cst, \
             tc.tile_pool(name="ps", bufs=2, space="PSUM") as ps, \
             tc.tile_pool(name="psg", bufs=3, space="PSUM") as psg, \
             tc.tile_pool(name="sb", bufs=3) as sb:

            # ---------- constants / weights in ----------
            ident = cst.tile([128, 128], b16)
            make_identity(nc, ident[:])
            idx_sb = cst.tile([128, 32], i32)
            nc.sync.dma_start(out=idx_sb[:], in_=idx_d[:])
            sml = cst.tile([32, 35], f32)
            nc.sync.dma_start(out=sml[:], in_=sml_d[:])
            ihA, ihB, Whh, Wo = {}, {}, {}, {}
            for di in range(2):
                for j in range(4):
                    wa = cst.tile([128, 128], b16, tag=f"ihA{di}{j}")
                    nc.sync.dma_start(out=wa[:], in_=ihA_d[di, j])
                    ihA[di, j] = wa
                    wb = cst.tile([65, 128], b16, tag=f"ihB{di}{j}")
                    nc.sync.dma_start(out=wb[:], in_=ihB_d[di, j])
                    ihB[di, j] = wb
                    wh = cst.tile([128, 128], b16, tag=f"whh{di}{j}")
                    nc.sync.dma_start(out=wh[:], in_=whh_d[di, j])
                    Whh[di, j] = wh
                wo = cst.tile([128, 32], b16, tag=f"wo{di}")
                nc.sync.dma_start(out=wo[:], in_=wo_d[di])
                Wo[di] = wo

            # ---------- embedding gather + transpose ----------
            xT_hi = cst.tile([128, NF], b16)   # word dims 0:128, x^T
            xT_lo = cst.tile([65, NF], b16)    # char dims + ones row
            nc.gpsimd.memset(xT_lo[64:65, :], 1.0)
            for k in range(KT):
                xw = sb.tile([128, DW], b16, tag="xw")
                nc.gpsimd.indirect_dma_start(
                    out=xw[:], out_offset=None, in_=wemb_d[:],
                    in_offset=bass.IndirectOffsetOnAxis(
                        ap=idx_sb[:, k:k + 1], axis=0))
                tw = ps.tile([128, 128], b16, tag="big")
                nc.tensor.transpose(out=tw[:], in_=xw[:], identity=ident[:])
                nc.vector.tensor_copy(out=xT_hi[:, k * 128:(k + 1) * 128],
                                      in_=tw[:])
                xc = sb.tile([128, DC], b16, tag="xc")
                nc.gpsimd.indirect_dma_start(
                    out=xc[:], out_offset=None, in_=cemb_d[:],
                    in_offset=bass.IndirectOffsetOnAxis(
                        ap=idx_sb[:, 16 + k:17 + k], axis=0))
                tcp = ps.tile([64, 128], b16, tag="big")
                nc.tensor.transpose(out=tcp[:], in_=xc[:], identity=ident[:])
                nc.scalar.copy(out=xT_lo[0:64, k * 128:(k + 1) * 128],
                               in_=tcp[:])

            # ---------- xp GEMM:  xp^T[(chunk j), t, b] per dir ----------
            # xp buffer layout per dir: col = t*16 + j*4 + b  (bwd: t reversed)
            xp = {}
            for di in range(2):
                xpt = cst.tile([128, NT * 16], b16, tag=f"xp{di}")
                xp[di] = xpt
            for di in range(2):
                for j in range(4):
                    for b in range(BL):
                        gps = ps.tile([128, NT], f32, tag="big")
                        nc.tensor.matmul(gps[:], ihA[di, j][:],
                                         xT_hi[:, b * NT:(b + 1) * NT],
                                         start=True, stop=False)
                        nc.tensor.matmul(gps[:], ihB[di, j][:],
                                         xT_lo[:, b * NT:(b + 1) * NT],
                                         start=False, stop=True)
                        dst = xp[di][:].rearrange(
                            "p (t x) -> p t x", x=16)[:, :, j * 4 + b:j * 4 + b + 1]
                        if (j + di) % 2 == 0:
                            nc.vector.tensor_copy(out=dst, in_=gps[:].rearrange(
                                "p (t o) -> p t o", o=1))
                        else:
                            nc.scalar.copy(out=dst, in_=gps[:].rearrange(
                                "p (t o) -> p t o", o=1))

            # ---------- BiLSTM chains (lockstep f/b) ----------
            hh = {}
            for di in range(2):
                hht = cst.tile([128, NT * 4], b16, tag=f"hh{di}")
                hh[di] = hht
            h0 = cst.tile([128, 8], b16)
            nc.gpsimd.memset(h0[:], 0.0)
            cst8, acc = {}, {}
            for di in range(2):
                c_ = cst.tile([128, 4], f32, tag=f"c{di}")
                a_ = cst.tile([128, 1], f32, tag=f"acc{di}")
                nc.gpsimd.memset(c_[:], 0.0)
                cst8[di], acc[di] = c_, a_

            gsave = {}
            for di in range(2):
                g_ps = psg.tile([128, 16], f32, tag=f"g{di}")
                nc.tensor.matmul(g_ps[:], ident[:], xp[di][:, 0:16]
                                 if di == 0 else
                                 xp[di][:, (NT - 1) * 16:NT * 16],
                                 start=True, stop=False, skip_group_check=True)
                gsave[di] = g_ps
            for s in range(NT):
                for di in range(2):
                    # hh slot for this step (bwd history stored in natural t)
                    wr = s if di == 0 else (NT - 1 - s)
                    g_ps = gsave[di]
                    if s == 0:
                        hp = h0[:, di * 4:(di + 1) * 4]
                    else:
                        pw = (s - 1) if di == 0 else (NT - s)
                        hp = hh[di][:, pw * 4:(pw + 1) * 4]
                    for j in range(4):
                        nc.tensor.matmul(g_ps[:, j * 4:(j + 1) * 4],
                                         Whh[di, j][:], hp,
                                         start=False, stop=(j == 3),
                                         skip_group_check=True)
                    if s + 1 < NT:
                        xs2 = (s + 1) if di == 0 else (NT - 2 - s)
                        g_nx = psg.tile([128, 16], f32, tag=f"g{di}")
                        nc.tensor.matmul(g_nx[:], ident[:],
                                         xp[di][:, xs2 * 16:(xs2 + 1) * 16],
                                         start=True, stop=False,
                                         skip_group_check=True)
                        gsave[di] = g_nx
                    v = sb.tile([128, 16], f32, tag=f"v{di}")
                    nc.scalar.activation(v[:], g_ps[:], Tanh)
                    p_ = sb.tile([128, 4], f32, tag=f"p{di}")
                    nc.vector.affine_mul_reduce(
                        out=p_[:], accum_out=acc[di][:],
                        in0=v[:, 4:8], in1=cst8[di][:], scale=0.5, bias=0.5)
                    q_ = sb.tile([128, 4], f32, tag=f"q{di}")
                    nc.vector.affine_mul_reduce(
                        out=q_[:], accum_out=acc[di][:],
                        in0=v[:, 0:4], in1=v[:, 12:16], scale=0.5, bias=0.5)
                    nc.vector.tensor_add(out=cst8[di][:], in0=p_[:],
                                         in1=q_[:])
                    t_ = sb.tile([128, 4], f32, tag=f"t{di}")
                    nc.scalar.activation(t_[:], cst8[di][:], Tanh)
                    nc.vector.affine_mul_reduce(
                        out=hh[di][:, wr * 4:(wr + 1) * 4], accum_out=acc[di][:],
                        in0=v[:, 8:12], in1=t_[:], scale=0.5, bias=0.5)

            # ---------- emissions ----------
            emit_sb = cst.tile([32, NF], f32)
            em = cst.tile([32, NF], f32)
            CH = min(512, NF)
            for c0 in range(0, NF, CH):
                sl = slice(c0, c0 + CH)
                pse = ps.tile([32, CH], f32, tag="big")
                nc.tensor.matmul(pse[:], Wo[0][:], hh[0][:, sl],
                                 start=True, stop=False)
                nc.tensor.matmul(pse[:], Wo[1][:], hh[1][:, sl],
                                 start=False, stop=True)
                nc.vector.tensor_copy(out=emit_sb[:, sl], in_=pse[:])
                if c0 == 0:
                    nc.scalar.activation(em[:, 0:4], pse[:, 0:4], Exp,
                                         bias=sml[:, 33:34])
                    nc.scalar.activation(em[:, 4:CH], pse[:, 4:], Exp,
                                         bias=sml[:, 32:33])
                else:
                    nc.scalar.activation(em[:, sl], pse[:], Exp,
                                         bias=sml[:, 32:33])

            # ---------- gold emit score:  sum_t emit[y] per b ----------
            oneh = cst.tile([32, NF], f32)
            nc.sync.dma_start(out=oneh[:], in_=oneh_d[:])
            es = cst.tile([32, BL], f32)
            junk = cst.tile([32, NT], f32)
            ev = emit_sb[:].rearrange("p (t b) -> p b t", b=BL)
            ov = oneh[:].rearrange("p (t b) -> p b t", b=BL)
            for b in range(BL):
                nc.vector.affine_mul_reduce(
                    out=junk[:], accum_out=es[:, b:b + 1],
                    in0=ev[:, b], in1=ov[:, b], scale=1.0, bias=0.0)

            # ---------- CRF forward scan (prob domain, bf16 matmul) ----------
            Eb = cst.tile([32, 32], b16)
            nc.vector.tensor_copy(out=Eb[:], in_=sml[:, 0:32])
            aT = cst.tile([32, 4], b16)
            nc.vector.tensor_copy(out=aT[:], in_=em[:, 0:4])
            for t in range(1, NT):
                ca = psg.tile([32, 4], f32, tag="g0")
                nc.tensor.matmul(ca[:], Eb[:], aT[:],
                                 start=True, stop=True)
                a2 = cst.tile([32, 4], b16, tag=f"aT{t % 2}")
                nc.vector.tensor_mul(out=a2[:], in0=ca[:],
                                     in1=em[:, t * 4:(t + 1) * 4])
                aT = a2

            # ---------- outputs: vsum + emit score ----------
            vfin = cst.tile([32, 4], f32)
            nc.vector.tensor_mul(out=vfin[:], in0=aT[:],
                                 in1=sml[:, 34:35].to_broadcast([32, 4]))
            ones = cst.tile([32, 1], f32)
            nc.gpsimd.memset(ones[:], 1.0)
            outp0 = psg.tile([1, BL], f32, tag="g0")
            nc.tensor.matmul(outp0[:], ones[:], vfin[:], start=True, stop=True)
            outp1 = psg.tile([1, BL], f32, tag="g0")
            nc.tensor.matmul(outp1[:], ones[:], es[:], start=True, stop=True)
            out_sb = cst.tile([1, 2 * BL], f32)
            nc.vector.tensor_copy(out=out_sb[:, 0:BL], in_=outp0[:])
            nc.vector.tensor_copy(out=out_sb[:, BL:], in_=outp1[:])
            nc.gpsimd.dma_start(out=out_d[:], in_=out_sb[:])

    nc.compile()
    return nc


def _get_nc():
    if "nc" not in _CACHE:
        import concourse.bacc as bacc
        _CACHE["nc"] = _build(bacc)
    return _CACHE["nc"]


def _prep_weights(W_ih, b, W_hh, W_out):
    """Per-direction weight prep: gate order (i,f,o,g), i/f/o pre-scaled by
    0.5 for the tanh-half-angle sigmoid."""
    Wi = W_ih.astype(np.float64).copy()
    Wh = W_hh.astype(np.float64).copy()
    bb = b.astype(np.float64).copy()
    # torch gate order i,f,g,o -> ours i,f,o,g
    perm = np.concatenate([np.arange(0, 128), np.arange(128, 256),
                           np.arange(384, 512), np.arange(256, 384)])
    Wi, Wh, bb = Wi[perm], Wh[perm], bb[perm]
    scale = np.ones((512, 1)); scale[:384] = 0.5
    Wi *= scale; Wh *= scale; bb *= scale[:, 0]
    # ihA[j] = Wi[j*128:(j+1)*128, 0:128].T ; ihB[j] = [Wi[:,128:192] | b].T
    ihA = np.stack([Wi[j * 128:(j + 1) * 128, 0:128].T for j in range(4)])
    ihB = np.stack([np.concatenate([Wi[j * 128:(j + 1) * 128, 128:192],
                                    bb[j * 128:(j + 1) * 128, None]], 1).T
                    for j in range(4)])
    whh = np.stack([Wh[j * 128:(j + 1) * 128, :].T for j in range(4)])
    return (ihA.astype(bf16), ihB.astype(bf16), whh.astype(bf16))


def _run_device(word, char, wordemb, charemb, W_ih_f, W_hh_f, b_f,
                W_ih_b, W_hh_b, b_b, W_out, b_out, trans, start, end, y):
    from concourse.bass_utils import run_bass_kernel_spmd
    nc = _get_nc()

    ihA_f, ihB_f, whh_f = _prep_weights(W_ih_f, b_f, W_hh_f, W_out)
    ihA_b, ihB_b, whh_b = _prep_weights(W_ih_b, b_b, W_hh_b, W_out)
    ihA = np.stack([ihA_f, ihA_b]); ihB = np.stack([ihB_f, ihB_b])
    whh = np.stack([whh_f, whh_b])
    wo = np.stack([W_out[:, 0:128].T.astype(bf16),
                   W_out[:, 128:256].T.astype(bf16)])
    sml = np.zeros((32, 35), np.float32)
    sml[:, 0:32] = np.exp(trans.astype(np.float64) / 1.0).astype(np.float32) / 32.0
    sml[:, 32] = b_out
    sml[:, 33] = start + b_out
    sml[:, 34] = np.exp(end)
    wemb = wordemb.astype(bf16)
    cemb = charemb.astype(bf16)

    in_maps = []
    for c in range(NCORES):
        wl = word[c * BL:(c + 1) * BL, :NT]          # [4, NT]
        cl = char[c * BL:(c + 1) * BL, :NT]
        yl = y[c * BL:(c + 1) * BL, :NT]
        # column-major per gather call: call k covers flat (b-major) 128k..
        iw = wl.reshape(-1)          # n = b*NT + t
        ic = cl.reshape(-1)
        idx = np.zeros((128, 32), np.int32)
        for k in range(NT * BL // 128):
            idx[:, k] = iw[k * 128:(k + 1) * 128]
            idx[:, 16 + k] = ic[k * 128:(k + 1) * 128]
        oneh = np.zeros((32, NT * BL), np.float32)
        tt, bb2 = np.meshgrid(np.arange(NT), np.arange(BL), indexing="ij")
        oneh[yl.T.reshape(-1), (tt * BL + bb2).reshape(-1)] = 1.0
        in_maps.append(dict(wemb=wemb, cemb=cemb, idx=idx, ihA=ihA, ihB=ihB,
                            whh=whh, wo=wo, sml=sml, oneh=oneh))
    res = run_bass_kernel_spmd(nc, in_maps, list(range(NCORES)),
                               **_CACHE.get("run_kwargs", {}))
    _CACHE["last_res"] = res
    outs = [np.asarray(res.results[c]["out"], np.float64).reshape(2, BL)
            for c in range(NCORES)]
    return outs


def kernel(**inputs):
    word = np.asarray(inputs["word"]).astype(np.int64)
    char = np.asarray(inputs["char"]).astype(np.int64)
    y = np.asarray(inputs["y"]).astype(np.int64)
    wordemb = np.asarray(inputs["wordemb"], np.float32)
    charemb = np.asarray(inputs["charemb"], np.float32)
    W_ih_f = np.asarray(inputs["W_ih_f"], np.float32)
    W_hh_f = np.asarray(inputs["W_hh_f"], np.float32)
    b_f = np.asarray(inputs["b_f"], np.float32)
    W_ih_b = np.asarray(inputs["W_ih_b"], np.float32)
    W_hh_b = np.asarray(inputs["W_hh_b"], np.float32)
    b_b = np.asarray(inputs["b_b"], np.float32)
    W_out = np.asarray(inputs["W_out"], np.float32)
    b_out = np.asarray(inputs["b_out"], np.float32)
    trans = np.asarray(inputs["trans"], np.float32)
    start = np.asarray(inputs["start"], np.float32)
    end = np.asarray(inputs["end"], np.float32)

    mask = (char > 0)
    if not mask.all():
        return _host_fallback(**inputs)

    outs = _run_device(word, char, wordemb, charemb, W_ih_f, W_hh_f, b_f,
                       W_ih_b, W_hh_b, b_b, W_out, b_out, trans, start, end, y)

    total = 0.0
    tr64 = trans.astype(np.float64)
    for c in range(NCORES):
        vsum = outs[c][0]          # [4]
        es_dev = outs[c][1]        # [4] sum_t (W_out h2)[y]
        yl = y[c * BL:(c + 1) * BL]
        logZ = np.log(vsum) + (T - 1) * np.log(32.0)
        bout_sum = b_out.astype(np.float64)[yl].sum(1)
        trans_sc = tr64[yl[:, :-1], yl[:, 1:]].sum(1)
        gold = (start.astype(np.float64)[yl[:, 0]] + es_dev + bout_sum +
                trans_sc + end.astype(np.float64)[yl[:, -1]])
        total += np.sum(logZ - gold)
    return np.float32(total)


def _host_fallback(**inputs):
    """Pure numpy reference path (used only if masking assumptions break)."""
    word = np.asarray(inputs["word"]).astype(np.int64)
    char = np.asarray(inputs["char"]).astype(np.int64)
    y = np.asarray(inputs["y"]).astype(np.int64)
    wordemb = np.asarray(inputs["wordemb"], np.float32)
    charemb = np.asarray(inputs["charemb"], np.float32)
    W_out = np.asarray(inputs["W_out"], np.float32)
    b_out = np.asarray(inputs["b_out"], np.float32)
    trans = np.asarray(inputs["trans"], np.float32)
    start = np.asarray(inputs["start"], np.float32)
    end = np.asarray(inputs["end"], np.float32)

    def sig(x):
        return 1.0 / (1.0 + np.exp(-x))

    def lstm(xp, W_hh):
        h = np.zeros((B, H), np.float32)
        c = np.zeros((B, H), np.float32)
        hs = np.empty((T, B, H), np.float32)
        WT = W_hh.T
        for t in range(T):
            g = xp[t] + h @ WT
            i = sig(g[:, :H]); f = sig(g[:, H:2 * H])
            gg = np.tanh(g[:, 2 * H:3 * H]); o = sig(g[:, 3 * H:])
            c = f * c + i * gg
            h = o * np.tanh(c)
            hs[t] = h
        return hs

    mask = (char > 0).astype(np.float32)
    x = np.concatenate([wordemb[word], charemb[char]], -1)
    flat = x.reshape(-1, D)
    pf = (flat @ np.asarray(inputs["W_ih_f"], np.float32).T +
          np.asarray(inputs["b_f"], np.float32)).reshape(B, T, G)
    pb = (flat @ np.asarray(inputs["W_ih_b"], np.float32).T +
          np.asarray(inputs["b_b"], np.float32)).reshape(B, T, G)
    h_f = lstm(pf.transpose(1, 0, 2), np.asarray(inputs["W_hh_f"], np.float32))
    h_b = lstm(pb.transpose(1, 0, 2)[::-1],
               np.asarray(inputs["W_hh_b"], np.float32))[::-1]
    h = np.concatenate([h_f, h_b], -1).transpose(1, 0, 2)
    emit = (h.reshape(-1, 2 * H) @ W_out.T + b_out).reshape(B, T, L)
    emit = emit * mask[:, :, None]

    def lse(xx, axis):
        m = np.max(xx, axis=axis, keepdims=True)
        return np.squeeze(m, axis) + np.log(np.sum(np.exp(xx - m), axis=axis))

    alpha = start + emit[:, 0]
    for t in range(1, T):
        new = lse(alpha[:, :, None] + trans[None], 1) + emit[:, t]
        alpha = np.where(mask[:, t:t + 1] > 0, new, alpha)
    logZ = lse(alpha + end[None], 1)
    emit_sc = (np.take_along_axis(emit, y[:, :, None], 2)[..., 0] * mask).sum(1)
    trans_sc = (trans[y[:, :-1], y[:, 1:]] * mask[:, 1:]).sum(1)
    last = mask.sum(1).astype(np.int64) - 1
    y_last = y[np.arange(B), last]
    gold = start[y[:, 0]] + emit_sc + trans_sc + end[y_last]
    return np.float32(np.sum(logZ - gold))
